# revision 16
# baseline (speedup 1.0000x reference)
"""Trainium2 Bass kernel for AmacrineCellFeedback.

Computes out = sigmoid(slope * (rowsum(K * (W @ x)) - offset)) for
W (8192, 8192), x (8192, 52), K (8192, 52), slope/offset (8192,).

Strategy: pure tensor parallelism over the AC dim — 8 cores, 1024 AC rows
each, no communication. Per core the dominant cost is streaming the
W shard from HBM; W and x are converted to bf16 on the host (halves DMA
bytes, 1 cycle/row matmul) and W is prepacked so each per-core DMA is a
fully sequential HBM read. Matmul layout: lhsT = W^T tile (jBC x iAC,
stationary), rhs = x chunk (jBC x T, moving), PSUM accumulates y (iAC x T)
over the 64 jBC chunks. Epilogue per 128-row tile: fused multiply+reduce
with K on VectorE, then Sigmoid on ScalarE with per-partition scale/bias.
"""

import os
import sys

import numpy as np

for _p in ("/opt/trn_rl_repo", "/opt/trn_rl_repo/concourse"):
    if _p not in sys.path:
        sys.path.insert(0, _p)

N_AC = 8192
N_BC = 8192
T = 52
N_CORES = 8
AC_PER_CORE = N_AC // N_CORES  # 1024
N_ITILE = AC_PER_CORE // 128   # 8
N_JCHUNK = N_BC // 128         # 64

LAST_EXEC_NS = None
LAST_TRACE_DIR = None
_CACHE = {}


def _install_trace_shim():
    """Enable NTFF profiling under axon in images missing antenv.axon_hooks.

    Only used when BASS_KERNEL_TRACE=1 (local perf iteration); the grading
    path never calls this.
    """
    import types

    if "antenv.axon_hooks" not in sys.modules:
        mod = types.ModuleType("antenv.axon_hooks")
        mod._hook = None

        def set_axon_ntff_profile_hook(h):
            mod._hook = h

        def get_axon_ntff_profile_hook():
            return mod._hook

        mod.set_axon_ntff_profile_hook = set_axon_ntff_profile_hook
        mod.get_axon_ntff_profile_hook = get_axon_ntff_profile_hook
        sys.modules["antenv.axon_hooks"] = mod
        try:
            import antenv
            antenv.axon_hooks = mod
        except ImportError:
            pass
    shim = sys.modules["antenv.axon_hooks"]
    if shim.get_axon_ntff_profile_hook() is None:
        from trn_agent_boot.trn_boot import _ntff_profile_via_ctypes
        shim.set_axon_ntff_profile_hook(
            _ntff_profile_via_ctypes("/opt/axon/libaxon_pjrt.so")
        )
    import concourse.bass_utils as bu
    bu.upload_artifacts = lambda tmpdir: f"file://{tmpdir}"


def _build_nc():
    import concourse.bacc as bacc
    import concourse.bass as bass
    from concourse import mybir, tile

    bf16 = mybir.dt.bfloat16
    f32 = mybir.dt.float32

    nc = bacc.Bacc("TRN2", target_bir_lowering=False, debug=False)

    wt = nc.declare_dram_parameter("wt", [N_ITILE, 128, N_JCHUNK * 128], bf16, isOutput=False)
    xp = nc.declare_dram_parameter("xp", [128, N_JCHUNK * T], bf16, isOutput=False)
    kp = nc.declare_dram_parameter("kp", [128, N_ITILE * T], f32, isOutput=False)
    sl = nc.declare_dram_parameter("sl", [128, N_ITILE], f32, isOutput=False)
    nb = nc.declare_dram_parameter("nb", [128, N_ITILE], f32, isOutput=False)
    out = nc.declare_dram_parameter("out", [128, N_ITILE], f32, isOutput=True)

    NJH = N_JCHUNK // 2  # 32 j-chunks per half-DMA

    with tile.TileContext(nc) as tc:
        with (
            tc.tile_pool(name="const", bufs=1) as cp,
            tc.tile_pool(name="w", bufs=6) as wp,
            tc.tile_pool(name="ep", bufs=2) as ep,
            tc.tile_pool(name="ps", bufs=4, space=bass.MemorySpace.PSUM) as pp,
        ):
            # x heads the sync HWDGE ring — every matmul needs it, and
            # within a ring DMAs drain in order, so nothing overtakes it.
            # k/slope/bias ride the SWDGE (gpsimd) ring; they are not
            # needed until the first epilogue (~16 us in).
            x_sb = cp.tile([128, N_JCHUNK * T], bf16)
            nc.sync.dma_start(out=x_sb[:], in_=xp[:])

            def emit_w_dmas(it):
                if it < N_ITILE - 1:
                    pa = wp.tile([128, NJH * 128], bf16, tag="wa")
                    pb = wp.tile([128, NJH * 128], bf16, tag="wb")
                    nc.sync.dma_start(out=pa[:], in_=wt[it, :, 0:NJH * 128])
                    nc.scalar.dma_start(out=pb[:], in_=wt[it, :, NJH * 128:])
                    return [(pa, 0, NJH), (pb, NJH, N_JCHUNK)]
                # Last tile: four even 512 KB quarters so the final chunk
                # to arrive is small and PE's tail after it is short.
                qs = []
                for qi in range(4):
                    q = wp.tile([128, 16 * 128], bf16, tag=f"wq{qi}")
                    eng = nc.sync if qi % 2 == 0 else nc.scalar
                    eng.dma_start(out=q[:], in_=wt[it, :, qi * 16 * 128:(qi + 1) * 16 * 128])
                    qs.append((q, qi * 16, (qi + 1) * 16))
                return qs

            w_pieces = [emit_w_dmas(0), emit_w_dmas(1)]

            k_sb = cp.tile([128, N_ITILE * T], f32)
            nc.gpsimd.dma_start(out=k_sb[:], in_=kp[:])
            sl_sb = cp.tile([128, N_ITILE], f32)
            nc.gpsimd.dma_start(out=sl_sb[:], in_=sl[:])
            nb_sb = cp.tile([128, N_ITILE], f32)
            nc.gpsimd.dma_start(out=nb_sb[:], in_=nb[:])
            o_sb = cp.tile([128, N_ITILE], f32)

            for it in range(N_ITILE):
                pieces = w_pieces[it]
                if it + 2 < N_ITILE:
                    w_pieces.append(emit_w_dmas(it + 2))
                ps = pp.tile([128, T], f32)
                for piece, j0, j1 in pieces:
                    for j in range(j0, j1):
                        jj = j - j0
                        nc.tensor.matmul(
                            ps[:],
                            piece[:, jj * 128:(jj + 1) * 128],
                            x_sb[:, j * T:(j + 1) * T],
                            start=(j == 0),
                            stop=(j == N_JCHUNK - 1),
                        )
                prod = ep.tile([128, T], f32)
                s = ep.tile([128, 1], f32)
                junk = ep.tile([128, T], f32)
                nc.vector.tensor_tensor(
                    out=prod[:],
                    in0=ps[:],
                    in1=k_sb[:, it * T:(it + 1) * T],
                    op=mybir.AluOpType.mult,
                )
                # reduce on ScalarE (Copy + accum_out) so reduce -> sigmoid
                # -> output DMA all stay on the scalar engine/ring.
                nc.scalar.activation(
                    out=junk[:],
                    in_=prod[:],
                    func=mybir.ActivationFunctionType.Copy,
                    accum_out=s[:],
                )
                nc.scalar.activation(
                    out=o_sb[:, it:it + 1],
                    in_=s[:],
                    func=mybir.ActivationFunctionType.Sigmoid,
                    bias=nb_sb[:, it:it + 1],
                    scale=sl_sb[:, it:it + 1],
                )
                # outputs for tiles 0..6 leave early; only the last 512 B
                # waits on the final sigmoid. Both ride the scalar ring,
                # which is idle by then.
                if it == N_ITILE - 2:
                    nc.scalar.dma_start(out=out[:, 0:N_ITILE - 1], in_=o_sb[:, 0:N_ITILE - 1])
                elif it == N_ITILE - 1:
                    nc.scalar.dma_start(out=out[:, N_ITILE - 1:], in_=o_sb[:, N_ITILE - 1:])

    nc.compile()
    return nc


def pack_inputs(x, bc_ac_weight, ac_kernel, ac_sigmoid_slope, ac_sigmoid_offset):
    import ml_dtypes

    bf16 = ml_dtypes.bfloat16

    x = np.asarray(x, dtype=np.float32)
    W = np.asarray(bc_ac_weight, dtype=np.float32)
    K = np.asarray(ac_kernel, dtype=np.float32)
    slope = np.asarray(ac_sigmoid_slope, dtype=np.float32)
    offset = np.asarray(ac_sigmoid_offset, dtype=np.float32)

    # x prepack: xp[p, jc*T + t] = x[jc*128 + p, t]  (shared by all cores)
    xp = np.ascontiguousarray(
        x.reshape(N_JCHUNK, 128, T).transpose(1, 0, 2).reshape(128, N_JCHUNK * T)
    ).astype(bf16)

    Wb = W.astype(bf16)
    negb = (-slope * offset).astype(np.float32)

    in_maps = []
    for c in range(N_CORES):
        lo, hi = c * AC_PER_CORE, (c + 1) * AC_PER_CORE
        # wt[it, p, jc*128 + ci] = W[lo + it*128 + ci, jc*128 + p]
        wc = np.ascontiguousarray(
            Wb[lo:hi]
            .reshape(N_ITILE, 128, N_JCHUNK, 128)
            .transpose(0, 3, 2, 1)
            .reshape(N_ITILE, 128, N_JCHUNK * 128)
        )
        kc = np.ascontiguousarray(
            K[lo:hi].reshape(N_ITILE, 128, T).transpose(1, 0, 2).reshape(128, N_ITILE * T)
        )
        slc = np.ascontiguousarray(slope[lo:hi].reshape(N_ITILE, 128).T)
        nbc = np.ascontiguousarray(negb[lo:hi].reshape(N_ITILE, 128).T)
        in_maps.append({"wt": wc, "xp": xp, "kp": kc, "sl": slc, "nb": nbc})
    return in_maps


def unpack_output(results):
    out = np.empty((N_AC,), dtype=np.float32)
    for c in range(N_CORES):
        o = np.asarray(results[c]["out"])  # (128, N_ITILE)
        out[c * AC_PER_CORE:(c + 1) * AC_PER_CORE] = o.T.reshape(AC_PER_CORE)
    return out


def kernel(x, bc_ac_weight, ac_kernel, ac_sigmoid_slope, ac_sigmoid_offset):
    global LAST_EXEC_NS
    from concourse.bass_utils import run_bass_kernel_spmd

    in_maps = pack_inputs(x, bc_ac_weight, ac_kernel, ac_sigmoid_slope, ac_sigmoid_offset)

    if "nc" not in _CACHE:
        _CACHE["nc"] = _build_nc()
    nc = _CACHE["nc"]

    trace = os.environ.get("BASS_KERNEL_TRACE", "0") == "1"
    if trace:
        global LAST_TRACE_DIR
        import tempfile
        _install_trace_shim()
        LAST_TRACE_DIR = tempfile.mkdtemp(prefix="bass_trace_")
        res = run_bass_kernel_spmd(
            nc, in_maps, core_ids=list(range(N_CORES)), trace=True,
            tmpdir=LAST_TRACE_DIR,
        )
    else:
        res = run_bass_kernel_spmd(nc, in_maps, core_ids=list(range(N_CORES)))
    LAST_EXEC_NS = res.exec_time_ns
    return unpack_output(res.results)


# revision 18
# speedup vs baseline: 1.0743x; 1.0743x over previous
"""Trainium2 Bass kernel for AmacrineCellFeedback.

Computes out = sigmoid(slope * (rowsum(K * (W @ x)) - offset)) for
W (8192, 8192), x (8192, 52), K (8192, 52), slope/offset (8192,).

Strategy: pure tensor parallelism over the AC dim - 8 cores, 1024 AC rows
each, no communication. Per core the cost is HBM-streaming the W shard;
W and x are converted to bf16 on the host (halves DMA bytes, 1 cycle/row
matmul; measured output L2 err 9.5e-4) and W is prepacked into an
SBUF-mirrored layout so every DMA is a same-shape column-range copy.

Raw-bacc program (no TileContext), everything SBUF-resident (no buffer
reuse): weights stream over both HWDGE rings (sync + scalar engines) as
three 2 MB pair-DMAs plus halves, with the last tile as four 512 KB
quarters so PE's tail after the final chunk is short. x heads the sync
ring (in-order queue - nothing overtakes it); k/slope/bias ride SWDGE.
Matmul: lhsT = W^T tile (jBC x iAC, stationary), rhs = x chunk (jBC x T,
moving), PSUM bank per tile accumulates y over 64 j-chunks. Epilogue:
VectorE multiply by K, ScalarE Copy+accum reduce then Sigmoid with
per-partition scale/bias, split output DMAs (only the last 512 B waits
on the final sigmoid; no end-of-program wait - NRT quiesces DMA rings).
"""

import os
import sys

import numpy as np

for _p in ("/opt/trn_rl_repo", "/opt/trn_rl_repo/concourse"):
    if _p not in sys.path:
        sys.path.insert(0, _p)

N_AC = 8192
N_BC = 8192
T = 52
N_CORES = 8
AC_PER_CORE = N_AC // N_CORES  # 1024
N_ITILE = AC_PER_CORE // 128   # 8
N_JCHUNK = N_BC // 128         # 64
JB = N_JCHUNK * 128            # 8192 weight columns per tile block

LAST_EXEC_NS = None
LAST_TRACE_DIR = None
_CACHE = {}


def _install_trace_shim():
    """Enable NTFF profiling under axon in images missing antenv.axon_hooks.

    Only used when BASS_KERNEL_TRACE=1 (local perf iteration); the grading
    path never calls this.
    """
    import types

    if "antenv.axon_hooks" not in sys.modules:
        mod = types.ModuleType("antenv.axon_hooks")
        mod._hook = None

        def set_axon_ntff_profile_hook(h):
            mod._hook = h

        def get_axon_ntff_profile_hook():
            return mod._hook

        mod.set_axon_ntff_profile_hook = set_axon_ntff_profile_hook
        mod.get_axon_ntff_profile_hook = get_axon_ntff_profile_hook
        sys.modules["antenv.axon_hooks"] = mod
        try:
            import antenv
            antenv.axon_hooks = mod
        except ImportError:
            pass
    shim = sys.modules["antenv.axon_hooks"]
    if shim.get_axon_ntff_profile_hook() is None:
        from trn_agent_boot.trn_boot import _ntff_profile_via_ctypes
        shim.set_axon_ntff_profile_hook(
            _ntff_profile_via_ctypes("/opt/axon/libaxon_pjrt.so")
        )
    import concourse.bass_utils as bu
    bu.upload_artifacts = lambda tmpdir: f"file://{tmpdir}"


def build_nc(debug=False):
    import concourse.bacc as bacc
    import concourse.bass as bass
    from concourse import mybir

    bf16 = mybir.dt.bfloat16
    f32 = mybir.dt.float32
    NJH = N_JCHUNK // 2  # 32
    H = NJH * 128        # 4096 cols = half a tile

    nc = bacc.Bacc("TRN2", target_bir_lowering=False, debug=debug)

    wt = nc.declare_dram_parameter("wt", [128, N_ITILE, JB], bf16, isOutput=False)
    xp = nc.declare_dram_parameter("xp", [128, N_JCHUNK * T], bf16, isOutput=False)
    kp = nc.declare_dram_parameter("kp", [128, N_ITILE * T], f32, isOutput=False)
    sl = nc.declare_dram_parameter("sl", [128, N_ITILE], f32, isOutput=False)
    nbp = nc.declare_dram_parameter("nb", [128, N_ITILE], f32, isOutput=False)
    out = nc.declare_dram_parameter("out", [128, N_ITILE], f32, isOutput=True)

    x_sb = nc.alloc_sbuf_tensor("x_sb", [128, N_JCHUNK * T], bf16)
    k_sb = nc.alloc_sbuf_tensor("k_sb", [128, N_ITILE * T], f32)
    sl_sb = nc.alloc_sbuf_tensor("sl_sb", [128, N_ITILE], f32)
    nb_sb = nc.alloc_sbuf_tensor("nb_sb", [128, N_ITILE], f32)
    o_sb = nc.alloc_sbuf_tensor("o_sb", [128, N_ITILE], f32)
    w_all = nc.alloc_sbuf_tensor("w_all", [128, N_ITILE, JB], bf16)
    prod = [nc.alloc_sbuf_tensor(f"prod{t}", [128, T], f32) for t in range(N_ITILE)]
    junk = [nc.alloc_sbuf_tensor(f"junk{t}", [128, T], f32) for t in range(N_ITILE)]
    s_sb = [nc.alloc_sbuf_tensor(f"s{t}", [128, 1], f32) for t in range(N_ITILE)]
    ps = [nc.alloc_psum_tensor(f"ps{t}", [128, 512], f32) for t in range(N_ITILE)]

    # DMA piece lists per ring: (tile_range, col_range) in w layout terms.
    # sync:   x, A01, A23, A45, a6, q0, q2
    # scalar: B01, B23, B45, b6, q1, q3 (+ the two output DMAs)
    Q = 16 * 128  # 2048 cols = quarter of a tile
    sync_pieces = [((0, 2), (0, H)), ((2, 4), (0, H)), ((4, 6), (0, H)),
                   ((6, 7), (0, H)), ((7, 8), (0, Q)), ((7, 8), (2 * Q, 3 * Q))]
    scal_pieces = [((0, 2), (H, JB)), ((2, 4), (H, JB)), ((4, 6), (H, JB)),
                   ((6, 7), (H, JB)), ((7, 8), (Q, 2 * Q)), ((7, 8), (3 * Q, JB))]

    # PE waits: before the first matmul of (tile, jchunk) consult this map.
    pe_waits = {}
    for i, (ts, cr) in enumerate(sync_pieces):
        pe_waits[(ts[0], cr[0] // 128)] = ("a", i)
    for i, (ts, cr) in enumerate(scal_pieces):
        pe_waits[(ts[0], cr[0] // 128)] = ("b", i)

    from contextlib import ExitStack
    with ExitStack() as stack:
        s_x = stack.enter_context(nc.semaphore("s_x"))
        s_a = [stack.enter_context(nc.semaphore(f"s_a{i}")) for i in range(len(sync_pieces))]
        s_b = [stack.enter_context(nc.semaphore(f"s_b{i}")) for i in range(len(scal_pieces))]
        s_k = stack.enter_context(nc.semaphore("s_k"))
        s_cn = stack.enter_context(nc.semaphore("s_cn"))
        s_mm = stack.enter_context(nc.semaphore("s_mm"))
        s_dv = stack.enter_context(nc.semaphore("s_dv"))
        s_out = stack.enter_context(nc.semaphore("s_out"))
        block = stack.enter_context(nc.Block(no_gpsimd_drain=True))

        @block.sync
        def _(se: bass.BassEngine):
            se.dma_start(out=x_sb[:], in_=xp[:]).then_inc(s_x, 16)
            for i, ((t0, t1), (c0, c1)) in enumerate(sync_pieces):
                se.dma_start(
                    out=w_all[:, t0:t1, c0:c1], in_=wt[:, t0:t1, c0:c1]
                ).then_inc(s_a[i], 16)

        @block.scalar
        def _(se: bass.BassEngine):
            for i, ((t0, t1), (c0, c1)) in enumerate(scal_pieces):
                se.dma_start(
                    out=w_all[:, t0:t1, c0:c1], in_=wt[:, t0:t1, c0:c1]
                ).then_inc(s_b[i], 16)
            se.wait_ge(s_cn, 32)
            for t in range(N_ITILE):
                se.wait_ge(s_dv, t + 1)
                se.activation(
                    out=junk[t][:],
                    in_=prod[t][:],
                    func=mybir.ActivationFunctionType.Copy,
                    accum_out=s_sb[t][:],
                )
                se.drain()
                se.activation(
                    out=o_sb[:, t:t + 1],
                    in_=s_sb[t][:],
                    func=mybir.ActivationFunctionType.Sigmoid,
                    bias=nb_sb[:, t:t + 1],
                    scale=sl_sb[:, t:t + 1],
                )
                if t == N_ITILE - 3:
                    se.drain()
                    se.dma_start(out=out[:, 0:6], in_=o_sb[:, 0:6]).then_inc(s_out, 16)
                elif t == N_ITILE - 1:
                    se.drain()
                    se.dma_start(out=out[:, 6:8], in_=o_sb[:, 6:8]).then_inc(s_out, 16)

        @block.gpsimd
        def _(se: bass.BassEngine):
            se.dma_start(out=k_sb[:], in_=kp[:]).then_inc(s_k, 16)
            se.dma_start(out=sl_sb[:], in_=sl[:]).then_inc(s_cn, 16)
            se.dma_start(out=nb_sb[:], in_=nbp[:]).then_inc(s_cn, 16)

        @block.tensor
        def _(se: bass.BassEngine):
            se.wait_ge(s_x, 16)
            for t in range(N_ITILE):
                for j in range(N_JCHUNK):
                    w = pe_waits.get((t, j))
                    if w is not None:
                        kind, i = w
                        se.wait_ge(s_a[i] if kind == "a" else s_b[i], 16)
                    mm = se.matmul(
                        ps[t][:, 0:T],
                        w_all[:, t, j * 128:(j + 1) * 128],
                        x_sb[:, j * T:(j + 1) * T],
                        start=(j == 0),
                        stop=(j == N_JCHUNK - 1),
                    )
                    if j == N_JCHUNK - 1:
                        mm.then_inc(s_mm)

        @block.vector
        def _(se: bass.BassEngine):
            se.wait_ge(s_k, 16)
            for t in range(N_ITILE):
                se.wait_ge(s_mm, t + 1)
                se.tensor_tensor(
                    out=prod[t][:],
                    in0=ps[t][:, 0:T],
                    in1=k_sb[:, t * T:(t + 1) * T],
                    op=mybir.AluOpType.mult,
                ).then_inc(s_dv)

    nc.compile()
    return nc


def pack_inputs(x, bc_ac_weight, ac_kernel, ac_sigmoid_slope, ac_sigmoid_offset):
    import ml_dtypes

    bf16 = ml_dtypes.bfloat16

    x = np.asarray(x, dtype=np.float32)
    W = np.asarray(bc_ac_weight, dtype=np.float32)
    K = np.asarray(ac_kernel, dtype=np.float32)
    slope = np.asarray(ac_sigmoid_slope, dtype=np.float32)
    offset = np.asarray(ac_sigmoid_offset, dtype=np.float32)

    # x prepack: xp[p, jc*T + t] = x[jc*128 + p, t]  (shared by all cores)
    xp = np.ascontiguousarray(
        x.reshape(N_JCHUNK, 128, T).transpose(1, 0, 2).reshape(128, N_JCHUNK * T)
    ).astype(bf16)

    Wb = W.astype(bf16)
    negb = (-slope * offset).astype(np.float32)

    in_maps = []
    for c in range(N_CORES):
        lo, hi = c * AC_PER_CORE, (c + 1) * AC_PER_CORE
        # wt[p, it, jc*128 + ci] = W[lo + it*128 + ci, jc*128 + p]
        # (SBUF-mirrored layout: partition outermost)
        wc = np.ascontiguousarray(
            Wb[lo:hi]
            .reshape(N_ITILE, 128, N_JCHUNK, 128)
            .transpose(3, 0, 2, 1)
            .reshape(128, N_ITILE, JB)
        )
        kc = np.ascontiguousarray(
            K[lo:hi].reshape(N_ITILE, 128, T).transpose(1, 0, 2).reshape(128, N_ITILE * T)
        )
        slc = np.ascontiguousarray(slope[lo:hi].reshape(N_ITILE, 128).T)
        nbc = np.ascontiguousarray(negb[lo:hi].reshape(N_ITILE, 128).T)
        in_maps.append({"wt": wc, "xp": xp, "kp": kc, "sl": slc, "nb": nbc})
    return in_maps


def unpack_output(results):
    out = np.empty((N_AC,), dtype=np.float32)
    for c in range(N_CORES):
        o = np.asarray(results[c]["out"])  # (128, N_ITILE)
        out[c * AC_PER_CORE:(c + 1) * AC_PER_CORE] = o.T.reshape(AC_PER_CORE)
    return out


def kernel(x, bc_ac_weight, ac_kernel, ac_sigmoid_slope, ac_sigmoid_offset):
    global LAST_EXEC_NS, LAST_TRACE_DIR
    from concourse.bass_utils import run_bass_kernel_spmd

    in_maps = pack_inputs(x, bc_ac_weight, ac_kernel, ac_sigmoid_slope, ac_sigmoid_offset)

    if "nc" not in _CACHE:
        _CACHE["nc"] = build_nc()
    nc = _CACHE["nc"]

    trace = os.environ.get("BASS_KERNEL_TRACE", "0") == "1"
    if trace:
        import tempfile
        _install_trace_shim()
        LAST_TRACE_DIR = tempfile.mkdtemp(prefix="bass_trace_")
        res = run_bass_kernel_spmd(
            nc, in_maps, core_ids=list(range(N_CORES)), trace=True,
            tmpdir=LAST_TRACE_DIR,
        )
    else:
        res = run_bass_kernel_spmd(nc, in_maps, core_ids=list(range(N_CORES)))
    LAST_EXEC_NS = res.exec_time_ns
    return unpack_output(res.results)


# revision 19
# speedup vs baseline: 1.0822x; 1.0073x over previous
"""Trainium2 Bass kernel for AmacrineCellFeedback.

Computes out = sigmoid(slope * (rowsum(K * (W @ x)) - offset)) for
W (8192, 8192), x (8192, 52), K (8192, 52), slope/offset (8192,).

Strategy: pure tensor parallelism over the AC dim - 8 cores, 1024 AC rows
each, no communication. Per core the cost is HBM-streaming the W shard;
W and x are converted to bf16 on the host (halves DMA bytes, 1 cycle/row
matmul; measured output L2 err 9.5e-4) and W is prepacked into an
SBUF-mirrored layout so every DMA is a same-shape column-range copy.

Raw-bacc program (no TileContext), everything SBUF-resident (no buffer
reuse): weights stream over both HWDGE rings (sync + scalar engines) as
three 2 MB pair-DMAs plus halves, with the last tile as four 512 KB
quarters so PE's tail after the final chunk is short. x heads the sync
ring (in-order queue - nothing overtakes it); k/slope/bias ride SWDGE.
Matmul: lhsT = W^T tile (jBC x iAC, stationary), rhs = x chunk (jBC x T,
moving), PSUM bank per tile accumulates y over 64 j-chunks. Epilogue:
VectorE multiply by K, ScalarE Copy+accum reduce then Sigmoid with
per-partition scale/bias, split output DMAs (only the last 512 B waits
on the final sigmoid; no end-of-program wait - NRT quiesces DMA rings).
"""

import os
import sys

import numpy as np

for _p in ("/opt/trn_rl_repo", "/opt/trn_rl_repo/concourse"):
    if _p not in sys.path:
        sys.path.insert(0, _p)

N_AC = 8192
N_BC = 8192
T = 52
N_CORES = 8
AC_PER_CORE = N_AC // N_CORES  # 1024
N_ITILE = AC_PER_CORE // 128   # 8
N_JCHUNK = N_BC // 128         # 64
JB = N_JCHUNK * 128            # 8192 weight columns per tile block

LAST_EXEC_NS = None
LAST_TRACE_DIR = None
_CACHE = {}


def _install_trace_shim():
    """Enable NTFF profiling under axon in images missing antenv.axon_hooks.

    Only used when BASS_KERNEL_TRACE=1 (local perf iteration); the grading
    path never calls this.
    """
    import types

    if "antenv.axon_hooks" not in sys.modules:
        mod = types.ModuleType("antenv.axon_hooks")
        mod._hook = None

        def set_axon_ntff_profile_hook(h):
            mod._hook = h

        def get_axon_ntff_profile_hook():
            return mod._hook

        mod.set_axon_ntff_profile_hook = set_axon_ntff_profile_hook
        mod.get_axon_ntff_profile_hook = get_axon_ntff_profile_hook
        sys.modules["antenv.axon_hooks"] = mod
        try:
            import antenv
            antenv.axon_hooks = mod
        except ImportError:
            pass
    shim = sys.modules["antenv.axon_hooks"]
    if shim.get_axon_ntff_profile_hook() is None:
        from trn_agent_boot.trn_boot import _ntff_profile_via_ctypes
        shim.set_axon_ntff_profile_hook(
            _ntff_profile_via_ctypes("/opt/axon/libaxon_pjrt.so")
        )
    import concourse.bass_utils as bu
    bu.upload_artifacts = lambda tmpdir: f"file://{tmpdir}"


def build_nc(debug=False):
    import concourse.bacc as bacc
    import concourse.bass as bass
    from concourse import mybir

    bf16 = mybir.dt.bfloat16
    f32 = mybir.dt.float32
    NJH = N_JCHUNK // 2  # 32
    H = NJH * 128        # 4096 cols = half a tile

    nc = bacc.Bacc("TRN2", target_bir_lowering=False, debug=debug)

    wt = nc.declare_dram_parameter("wt", [128, N_ITILE, JB], bf16, isOutput=False)
    xp = nc.declare_dram_parameter("xp", [128, N_JCHUNK * T], bf16, isOutput=False)
    kp = nc.declare_dram_parameter("kp", [128, N_ITILE * T], f32, isOutput=False)
    sl = nc.declare_dram_parameter("sl", [128, N_ITILE], f32, isOutput=False)
    nbp = nc.declare_dram_parameter("nb", [128, N_ITILE], f32, isOutput=False)
    out = nc.declare_dram_parameter("out", [128, N_ITILE], f32, isOutput=True)

    x_sb = nc.alloc_sbuf_tensor("x_sb", [128, N_JCHUNK * T], bf16)
    k_sb = nc.alloc_sbuf_tensor("k_sb", [128, N_ITILE * T], f32)
    sl_sb = nc.alloc_sbuf_tensor("sl_sb", [128, N_ITILE], f32)
    nb_sb = nc.alloc_sbuf_tensor("nb_sb", [128, N_ITILE], f32)
    o_sb = nc.alloc_sbuf_tensor("o_sb", [128, N_ITILE], f32)
    w_all = nc.alloc_sbuf_tensor("w_all", [128, N_ITILE, JB], bf16)
    prod = [nc.alloc_sbuf_tensor(f"prod{t}", [128, T], f32) for t in range(N_ITILE)]
    junk = [nc.alloc_sbuf_tensor(f"junk{t}", [128, T], f32) for t in range(N_ITILE)]
    s_sb = [nc.alloc_sbuf_tensor(f"s{t}", [128, 1], f32) for t in range(N_ITILE)]
    ps = [nc.alloc_psum_tensor(f"ps{t}", [128, 512], f32) for t in range(N_ITILE)]

    # DMA piece lists per ring: (tile_range, col_range) in w layout terms.
    # sync:   x, A01, A23, A45, a6, q0, q2
    # scalar: B01, B23, B45, b6, q1, q3 (+ the two output DMAs)
    Q = 16 * 128  # 2048 cols = quarter of a tile
    sync_pieces = [((0, 2), (0, H)), ((2, 4), (0, H)), ((4, 6), (0, H)),
                   ((6, 7), (0, H)), ((7, 8), (0, Q)), ((7, 8), (2 * Q, 3 * Q))]
    scal_pieces = [((0, 2), (H, JB)), ((2, 4), (H, JB)), ((4, 6), (H, JB)),
                   ((6, 7), (H, JB)), ((7, 8), (Q, 2 * Q)), ((7, 8), (3 * Q, JB))]

    # PE waits: before the first matmul of (tile, jchunk) consult this map.
    pe_waits = {}
    for i, (ts, cr) in enumerate(sync_pieces):
        pe_waits[(ts[0], cr[0] // 128)] = ("a", i)
    for i, (ts, cr) in enumerate(scal_pieces):
        pe_waits[(ts[0], cr[0] // 128)] = ("b", i)

    from contextlib import ExitStack
    with ExitStack() as stack:
        s_x = stack.enter_context(nc.semaphore("s_x"))
        s_a = [stack.enter_context(nc.semaphore(f"s_a{i}")) for i in range(len(sync_pieces))]
        s_b = [stack.enter_context(nc.semaphore(f"s_b{i}")) for i in range(len(scal_pieces))]
        s_k = stack.enter_context(nc.semaphore("s_k"))
        s_cn = stack.enter_context(nc.semaphore("s_cn"))
        s_mm = stack.enter_context(nc.semaphore("s_mm"))
        s_dv = stack.enter_context(nc.semaphore("s_dv"))
        s_out = stack.enter_context(nc.semaphore("s_out"))
        # Skip the end-of-program all-engine barrier: engines halt
        # independently (no cross-engine deps remain), the runtime resets
        # semaphores at each exec start, and NRT quiesces DMA rings.
        nc.all_engine_barrier = lambda *, sem_only=False: None
        block = stack.enter_context(nc.Block(no_gpsimd_drain=True))

        @block.sync
        def _(se: bass.BassEngine):
            se.dma_start(out=x_sb[:], in_=xp[:]).then_inc(s_x, 16)
            for i, ((t0, t1), (c0, c1)) in enumerate(sync_pieces):
                se.dma_start(
                    out=w_all[:, t0:t1, c0:c1], in_=wt[:, t0:t1, c0:c1]
                ).then_inc(s_a[i], 16)

        @block.scalar
        def _(se: bass.BassEngine):
            for i, ((t0, t1), (c0, c1)) in enumerate(scal_pieces):
                se.dma_start(
                    out=w_all[:, t0:t1, c0:c1], in_=wt[:, t0:t1, c0:c1]
                ).then_inc(s_b[i], 16)
            se.wait_ge(s_cn, 32)
            for t in range(N_ITILE):
                se.wait_ge(s_dv, t + 1)
                se.activation(
                    out=junk[t][:],
                    in_=prod[t][:],
                    func=mybir.ActivationFunctionType.Copy,
                    accum_out=s_sb[t][:],
                )
                se.drain()
                se.activation(
                    out=o_sb[:, t:t + 1],
                    in_=s_sb[t][:],
                    func=mybir.ActivationFunctionType.Sigmoid,
                    bias=nb_sb[:, t:t + 1],
                    scale=sl_sb[:, t:t + 1],
                )
                if t == N_ITILE - 3:
                    se.drain()
                    se.dma_start(out=out[:, 0:6], in_=o_sb[:, 0:6]).then_inc(s_out, 16)
                elif t == N_ITILE - 1:
                    se.drain()
                    se.dma_start(out=out[:, 6:8], in_=o_sb[:, 6:8]).then_inc(s_out, 16)

        @block.gpsimd
        def _(se: bass.BassEngine):
            se.dma_start(out=k_sb[:], in_=kp[:]).then_inc(s_k, 16)
            se.dma_start(out=sl_sb[:], in_=sl[:]).then_inc(s_cn, 16)
            se.dma_start(out=nb_sb[:], in_=nbp[:]).then_inc(s_cn, 16)

        @block.tensor
        def _(se: bass.BassEngine):
            se.wait_ge(s_x, 16)
            for t in range(N_ITILE):
                for j in range(N_JCHUNK):
                    w = pe_waits.get((t, j))
                    if w is not None:
                        kind, i = w
                        se.wait_ge(s_a[i] if kind == "a" else s_b[i], 16)
                    mm = se.matmul(
                        ps[t][:, 0:T],
                        w_all[:, t, j * 128:(j + 1) * 128],
                        x_sb[:, j * T:(j + 1) * T],
                        start=(j == 0),
                        stop=(j == N_JCHUNK - 1),
                    )
                    if j == N_JCHUNK - 1:
                        mm.then_inc(s_mm)

        @block.vector
        def _(se: bass.BassEngine):
            se.wait_ge(s_k, 16)
            for t in range(N_ITILE):
                se.wait_ge(s_mm, t + 1)
                se.tensor_tensor(
                    out=prod[t][:],
                    in0=ps[t][:, 0:T],
                    in1=k_sb[:, t * T:(t + 1) * T],
                    op=mybir.AluOpType.mult,
                ).then_inc(s_dv)

    nc.compile()
    return nc


def pack_inputs(x, bc_ac_weight, ac_kernel, ac_sigmoid_slope, ac_sigmoid_offset):
    import ml_dtypes

    bf16 = ml_dtypes.bfloat16

    x = np.asarray(x, dtype=np.float32)
    W = np.asarray(bc_ac_weight, dtype=np.float32)
    K = np.asarray(ac_kernel, dtype=np.float32)
    slope = np.asarray(ac_sigmoid_slope, dtype=np.float32)
    offset = np.asarray(ac_sigmoid_offset, dtype=np.float32)

    # x prepack: xp[p, jc*T + t] = x[jc*128 + p, t]  (shared by all cores)
    xp = np.ascontiguousarray(
        x.reshape(N_JCHUNK, 128, T).transpose(1, 0, 2).reshape(128, N_JCHUNK * T)
    ).astype(bf16)

    Wb = W.astype(bf16)
    negb = (-slope * offset).astype(np.float32)

    in_maps = []
    for c in range(N_CORES):
        lo, hi = c * AC_PER_CORE, (c + 1) * AC_PER_CORE
        # wt[p, it, jc*128 + ci] = W[lo + it*128 + ci, jc*128 + p]
        # (SBUF-mirrored layout: partition outermost)
        wc = np.ascontiguousarray(
            Wb[lo:hi]
            .reshape(N_ITILE, 128, N_JCHUNK, 128)
            .transpose(3, 0, 2, 1)
            .reshape(128, N_ITILE, JB)
        )
        kc = np.ascontiguousarray(
            K[lo:hi].reshape(N_ITILE, 128, T).transpose(1, 0, 2).reshape(128, N_ITILE * T)
        )
        slc = np.ascontiguousarray(slope[lo:hi].reshape(N_ITILE, 128).T)
        nbc = np.ascontiguousarray(negb[lo:hi].reshape(N_ITILE, 128).T)
        in_maps.append({"wt": wc, "xp": xp, "kp": kc, "sl": slc, "nb": nbc})
    return in_maps


def unpack_output(results):
    out = np.empty((N_AC,), dtype=np.float32)
    for c in range(N_CORES):
        o = np.asarray(results[c]["out"])  # (128, N_ITILE)
        out[c * AC_PER_CORE:(c + 1) * AC_PER_CORE] = o.T.reshape(AC_PER_CORE)
    return out


def kernel(x, bc_ac_weight, ac_kernel, ac_sigmoid_slope, ac_sigmoid_offset):
    global LAST_EXEC_NS, LAST_TRACE_DIR
    from concourse.bass_utils import run_bass_kernel_spmd

    in_maps = pack_inputs(x, bc_ac_weight, ac_kernel, ac_sigmoid_slope, ac_sigmoid_offset)

    if "nc" not in _CACHE:
        _CACHE["nc"] = build_nc()
    nc = _CACHE["nc"]

    trace = os.environ.get("BASS_KERNEL_TRACE", "0") == "1"
    if trace:
        import tempfile
        _install_trace_shim()
        LAST_TRACE_DIR = tempfile.mkdtemp(prefix="bass_trace_")
        res = run_bass_kernel_spmd(
            nc, in_maps, core_ids=list(range(N_CORES)), trace=True,
            tmpdir=LAST_TRACE_DIR,
        )
    else:
        res = run_bass_kernel_spmd(nc, in_maps, core_ids=list(range(N_CORES)))
    LAST_EXEC_NS = res.exec_time_ns
    return unpack_output(res.results)
